# revision 13
# baseline (speedup 1.0000x reference)
"""MemoryBank2D (vq_codebook) Trainium2 kernel.

reference semantics:
    feat = x.transpose(0,2,3,1).reshape(-1, C)           # [N, C]
    sim = l2norm(feat) @ l2norm(memory).T                # [N, K]
    attn = softmax(sim, axis=1)
    recon = (attn @ l2norm(memory)) -> [B, C, H, W]
    match_map = attn.max(axis=1)    -> [B, H, W]

Sharding: data-parallel over the batch axis. 16 batches / 8 cores = 2
batches per core; memory replicated. No collectives.

Per-core kernel (per 128-patch tile, patches on PSUM partitions):
  MM1: sim_raw[m, k] = sum_c x_bf16[c, m] * memT_n[c, k] (PSUM, fp32 accum)
       + gram matmul (rhs = x tile itself) whose diagonal is ||feat||^2
  softmax: rowmax on DVE; one ACT Exp pass computes
       attn = exp(sim_raw*rscale - rowmax*rscale)  (bf16) and Z via accum_out
       (match_map = 1/Z since max attn numerator is exp(0) = 1)
  MM2: PE-transpose attn -> attnT; recon_raw = attnT.T @ mem_n; scale by 1/Z
  output: PE-transpose recon to C-major so the DRAM write has 512B runs
"""

import numpy as np
from contextlib import ExitStack

import concourse.bass as bass
import concourse.tile as tile
from concourse import bacc, mybir
from concourse.bass import ds
from concourse.bass_utils import run_bass_kernel_spmd
from concourse.masks import make_identity
from concourse.hw_specs import get_activation_tables

F32 = mybir.dt.float32
BF16 = mybir.dt.bfloat16
F16 = mybir.dt.float16
AF = mybir.ActivationFunctionType
ALU = mybir.AluOpType
AX = mybir.AxisListType

# problem shapes (hardcoded per contract)
B, C, H, W = 16, 512, 64, 64
K = 1024
N_CORES = 8
B_CORE = B // N_CORES          # 2
HW = H * W                     # 4096
P = 128
CC = C // P                    # 4 contraction chunks
KC = K // P                    # 8 slot chunks
PT = 128                       # patches per tile
TILES_PER_B = HW // PT         # 32
N_TILES = B_CORE * TILES_PER_B  # 64


def _emit(tc, ctx, x_ap, mem_ap, recon_ap, match_ap, reps=1):
    nc = tc.nc

    # Pre-load the one ACT table set covering every function used here
    # (Ln, Exp, Copy, Square) so the table-load pass inserts no per-tile
    # reloads (~1.3us each).
    tables = list(get_activation_tables(nc.m.arch).keys())
    nc.scalar.add_instruction(mybir.InstLoadActFuncSet(
        name=nc.get_next_instruction_name(), ins=[], outs=[],
        act_func_set_id=tables.index("natural_log_exp_and_others")))

    const = ctx.enter_context(tc.tile_pool(name="const", bufs=1))
    identity_f32 = const.tile([P, P], F32)
    make_identity(nc, identity_f32[:])
    identity_f16 = const.tile([P, P], F16)
    make_identity(nc, identity_f16[:])
    mem_bf = const.tile([P, KC, C], F16)    # [slot_in_chunk, slot_chunk, C]
    memT_bf = const.tile([P, CC, K], F16)   # [c_in_chunk, c_chunk, slot]
    match_sb = const.tile([P, N_TILES], F32)

    # ---- preamble: normalize memory, build mem_bf and its transpose ----
    with tc.tile_pool(name="pre", bufs=1) as pre, \
         tc.tile_pool(name="pre_ps", bufs=2, space="PSUM") as pre_ps:
        mem_f32 = pre.tile([P, KC, C], F32)
        mem_re = mem_ap.rearrange("(j p) c -> p j c", p=P)
        for j in range(KC):
            nc.sync.dma_start(mem_f32[:, j], mem_re[:, j])
        r2m = pre.tile([P, KC], F32)
        sq_scr = pre.tile([P, C], F32)
        for j in range(KC):
            nc.scalar.activation(sq_scr[:], mem_f32[:, j], AF.Square,
                                 accum_out=r2m[:, j:j + 1])
        lnr = pre.tile([P, KC], F32)
        nc.scalar.activation(lnr[:], r2m[:], AF.Ln)
        rnrm = pre.tile([P, KC], F32)
        nc.scalar.activation(rnrm[:], lnr[:], AF.Exp, scale=-0.5)
        for j in range(KC):
            nc.vector.tensor_scalar_mul(mem_bf[:, j], mem_f32[:, j],
                                        rnrm[:, j:j + 1])
        for cc in range(CC):
            for j in range(KC):
                tp = pre_ps.tile([P, P], F16)
                nc.tensor.transpose(tp[:], mem_bf[:, j, ds(cc * P, P)],
                                    identity_f16[:])
                nc.scalar.copy(memT_bf[:, cc, ds(j * P, P)], tp[:])

    # ---- main loop pools ----
    x_pool = ctx.enter_context(tc.tile_pool(name="x", bufs=4))
    xbf_pool = ctx.enter_context(tc.tile_pool(name="xbf", bufs=4))
    attn_pool = ctx.enter_context(tc.tile_pool(name="attn", bufs=3))
    attnT_pool = ctx.enter_context(tc.tile_pool(name="attnT", bufs=3))
    out_pool = ctx.enter_context(tc.tile_pool(name="out", bufs=4))
    small = ctx.enter_context(tc.tile_pool(name="small", bufs=4))
    scr_pool = ctx.enter_context(tc.tile_pool(name="scr", bufs=2))
    # PSUM budget (8 banks): gram 1 + reconT 1 + sim 2x2 + attnT(f16) 1 + rc 1 = 8
    ps_gram = ctx.enter_context(tc.tile_pool(name="ps_gram", bufs=1, space="PSUM"))
    ps_sim = ctx.enter_context(tc.tile_pool(name="ps_sim", bufs=2, space="PSUM"))
    ps_trh = ctx.enter_context(tc.tile_pool(name="ps_trh", bufs=1, space="PSUM"))
    ps_rc = ctx.enter_context(tc.tile_pool(name="ps_rc", bufs=1, space="PSUM"))

    import contextlib
    loop_cm = (tc.For_i(0, reps, 1,
                        hint_engines=(mybir.EngineType.PE,
                                      mybir.EngineType.DVE,
                                      mybir.EngineType.Activation))
               if reps > 1 else contextlib.nullcontext())
    with loop_cm:
      for t in range(N_TILES):
        b, tb = divmod(t, TILES_PER_B)
        hw0 = tb * PT

        xt = x_pool.tile([P, CC, PT], F32, tag="xt")
        nc.sync.dma_start(
            xt[:], x_ap[b, :, ds(hw0, PT)].rearrange("(cc p) w -> p cc w", p=P))
        xbf = xbf_pool.tile([P, CC, PT], F16, tag="xbf")
        nc.gpsimd.tensor_copy(xbf[:], xt[:])

        # gram first: r2 -> rscale chain overlaps the sim matmuls below
        gram = ps_gram.tile([P, P], F32, tag="gram")
        for cc in range(CC):
            nc.tensor.matmul(gram[:], xbf[:, cc], xbf[:, cc],
                             start=(cc == 0), stop=(cc == CC - 1))
        r2 = small.tile([P, 1], F32, tag="r2")
        scr = scr_pool.tile([P, P], F32, tag="scr")
        nc.vector.tensor_tensor(scr[:], gram[:], identity_f32[:], ALU.mult)
        nc.vector.tensor_reduce(r2[:], scr[:], axis=AX.X, op=ALU.add)
        # rscale = r2^-0.5 = exp(-0.5 ln r2); Ln/Exp share one ACT table set
        lnr2 = small.tile([P, 1], F32, tag="lnr2")
        nc.scalar.activation(lnr2[:], r2[:], AF.Ln)
        rs = small.tile([P, 1], F32, tag="rs")
        nc.scalar.activation(rs[:], lnr2[:], AF.Exp, scale=-0.5)

        sim = ps_sim.tile([P, K], F32, tag="sim")
        for cc in range(CC):
            first, last = cc == 0, cc == CC - 1
            nc.tensor.matmul(sim[:, ds(0, 512)], xbf[:, cc],
                             memT_bf[:, cc, ds(0, 512)], start=first, stop=last)
            nc.tensor.matmul(sim[:, ds(512, 512)], xbf[:, cc],
                             memT_bf[:, cc, ds(512, 512)], start=first, stop=last)

        nmx = small.tile([P, 1], F32, tag="nmx")
        nc.vector.tensor_reduce(nmx[:], sim[:], axis=AX.X, op=ALU.max,
                                negate=True)
        negb = small.tile([P, 1], F32, tag="negb")
        nc.scalar.mul(negb[:], nmx[:], rs[:])

        attnN = attn_pool.tile([P, K], F16, tag="attnN")
        zsum = small.tile([P, 1], F32, tag="zsum")
        nc.scalar.activation(attnN[:], sim[:], AF.Exp,
                             bias=negb[:], scale=rs[:], accum_out=zsum[:])

        # transpose attn to slot-major: attnT[s, j, m]
        attnT = attnT_pool.tile([P, KC, PT], F16, tag="attnT")
        pat = ps_trh.tile([P, K], F16, tag="trh")
        for j in range(KC):
            nc.tensor.transpose(pat[:, ds(j * P, P)],
                                attnN[:, ds(j * P, P)], identity_f16[:])
        cp = pat.rearrange("p (j w) -> p j w", w=PT)
        nc.vector.tensor_copy(attnT[:, 0:4, :], cp[:, 0:4])
        nc.scalar.copy(attnT[:, 4:8, :], cp[:, 4:8])

        # MM2: recon_raw[m, c] = sum_s attnT[s, m] * mem_bf[s, c]
        prc = ps_rc.tile([P, C], F32, tag="prc")
        for j in range(KC):
            nc.tensor.matmul(prc[:], attnT[:, j], mem_bf[:, j],
                             start=(j == 0), stop=(j == KC - 1))

        # match_map tile = 1/Z ; also the recon row scale
        nc.vector.reciprocal(match_sb[:, t:t + 1], zsum[:])
        rc = out_pool.tile([P, C], F32, tag="rc")
        nc.vector.tensor_scalar_mul(rc[:], prc[:], match_sb[:, t:t + 1])

        # transpose recon to C-major and store (f32 to keep output precision)
        prt = ps_gram.tile([P, C], F32, tag="prt", name="prt")
        for cc in range(CC):
            nc.tensor.transpose(prt[:, ds(cc * P, P)], rc[:, ds(cc * P, P)],
                                identity_f32[:])
        rt = out_pool.tile([P, CC, P], F32, tag="rt")
        nc.vector.tensor_copy(rt[:], prt.rearrange("p (c w) -> p c w", w=P))
        nc.sync.dma_start(
            recon_ap[b, :, ds(hw0, PT)].rearrange("(cc p) w -> p cc w", p=P),
            rt[:])

    # ---- epilogue: transpose match_sb [P, N_TILES] -> [N_TILES, P], store ----
    mps = ps_rc.tile([P, C], F32, tag="prc", name="mps")
    nc.tensor.transpose(mps[:N_TILES, :P], match_sb[:], identity_f32[:])
    mt = out_pool.tile([N_TILES, P], F32, tag="mt")
    nc.vector.tensor_copy(mt[:], mps[:N_TILES, :P])
    for b in range(B_CORE):
        nc.sync.dma_start(
            match_ap[b].rearrange("(t p) -> t p", p=P),
            mt[ds(b * TILES_PER_B, TILES_PER_B), :])


_CACHE = {}


def build_nc(reps=1):
    if reps in _CACHE:
        return _CACHE[reps]
    nc = bacc.Bacc("TRN2", target_bir_lowering=False, debug=False,
                   num_devices=N_CORES)
    x_ap = nc.dram_tensor("x_s", [B_CORE, C, HW], F32, kind="ExternalInput").ap()
    mem_ap = nc.dram_tensor("mem", [K, C], F32, kind="ExternalInput").ap()
    recon_ap = nc.dram_tensor("recon_s", [B_CORE, C, HW], F32,
                              kind="ExternalOutput").ap()
    match_ap = nc.dram_tensor("match_s", [B_CORE, HW], F32,
                              kind="ExternalOutput").ap()
    with tile.TileContext(nc) as tc, ExitStack() as ctx:
        _emit(tc, ctx, x_ap, mem_ap, recon_ap, match_ap, reps=reps)
    nc.compile()
    _CACHE[reps] = nc
    return nc


def make_in_maps(x, memory):
    x = np.ascontiguousarray(np.asarray(x, dtype=np.float32)).reshape(
        N_CORES, B_CORE, C, HW)
    memory = np.ascontiguousarray(np.asarray(memory, dtype=np.float32))
    return [{"x_s": x[i], "mem": memory} for i in range(N_CORES)]


def kernel(x, memory):
    nc = build_nc()
    in_maps = make_in_maps(x, memory)
    res = run_bass_kernel_spmd(nc, in_maps, core_ids=list(range(N_CORES)))
    recon = np.stack([res.results[i]["recon_s"] for i in range(N_CORES)])
    match = np.stack([res.results[i]["match_s"] for i in range(N_CORES)])
    recon = recon.reshape(B, C, H, W).astype(np.float32)
    match = match.reshape(B, H, W).astype(np.float32)
    return recon, match


# revision 14
# speedup vs baseline: 1.0192x; 1.0192x over previous
"""MemoryBank2D (vq_codebook) Trainium2 kernel — 8-core data-parallel.

reference semantics:
    feat = x.transpose(0,2,3,1).reshape(-1, C)           # [N, C], N = B*H*W
    sim = l2norm(feat) @ l2norm(memory).T                # [N, K]
    attn = softmax(sim, axis=1)
    recon = (attn @ l2norm(memory)) -> [B, C, H, W]
    match_map = attn.max(axis=1)    -> [B, H, W]

Sharding: data-parallel over batch. 16 batches / 8 cores = 2 per core;
memory replicated; no collectives. Each core computes its recon/match
shard; the host concatenates.

Per-core kernel, per 128-patch tile (patches on PSUM partitions; x's
native [B, C, HW] layout IS the lhsT layout MM1 needs, so no input
transpose):
  - cast x tile f32->f16 on GPSIMD (idle engine)
  - gram matmul (rhs = x tile itself, f16) FIRST: its diagonal is
    ||feat||^2; the rscale = r2^-0.5 chain (DVE diag reduce -> ACT
    Ln -> ACT Exp(scale=-0.5)) overlaps the sim matmuls
  - MM1 f16: sim_raw = x^T @ memT_n, fp32 PSUM accumulation over 4
    C-chunks (2 N=512 matmuls per chunk)
  - softmax: negated rowmax on DVE; ONE ACT Exp pass computes
    attn = exp(sim_raw*rscale - rowmax*rscale) in f16 AND its row sum Z
    via accum_out. match_map = 1/Z exactly (max attn numerator = exp(0)).
    All ACT funcs (Ln/Exp/Copy) live in one pre-seeded table set
    ("natural_log_exp_and_others") => no per-tile 1.3us table reloads.
  - PE-transpose attn (f16, 1 cyc/row) into one PSUM bank -> attnT
  - MM2 f16: recon_raw = attnT.T @ mem_n (8 slot-chunks, N=512);
    scale rows by 1/Z; PE-transpose recon (f32) to C-major so the DRAM
    write has 512B-contiguous runs
PSUM budget exactly 8 banks: gram 1 + reconT 1 + sim 2x2 + attnT 1 + rc 1.

Measured (8 NeuronCores, axon): ~350 us per core-pass (slope of wall
time vs in-NEFF hardware-loop reps; cost model: 325 us, PE busy 289 us
vs 221 us pure-matmul floor). Accuracy vs f32 reference (absmax-rel):
recon ~1.7e-4, match ~5.7e-5.
"""

import numpy as np
from contextlib import ExitStack

import concourse.bass as bass
import concourse.tile as tile
from concourse import bacc, mybir
from concourse.bass import ds
from concourse.bass_utils import run_bass_kernel_spmd
from concourse.masks import make_identity
from concourse.hw_specs import get_activation_tables

F32 = mybir.dt.float32
BF16 = mybir.dt.bfloat16
F16 = mybir.dt.float16
AF = mybir.ActivationFunctionType
ALU = mybir.AluOpType
AX = mybir.AxisListType

# problem shapes (hardcoded per contract)
B, C, H, W = 16, 512, 64, 64
K = 1024
N_CORES = 8
B_CORE = B // N_CORES          # 2
HW = H * W                     # 4096
P = 128
CC = C // P                    # 4 contraction chunks
KC = K // P                    # 8 slot chunks
PT = 128                       # patches per tile
TILES_PER_B = HW // PT         # 32
N_TILES = B_CORE * TILES_PER_B  # 64


def _emit(tc, ctx, x_ap, mem_ap, recon_ap, match_ap, reps=1):
    nc = tc.nc

    # Pre-load the one ACT table set covering every function used here
    # (Ln, Exp, Copy, Square) so the table-load pass inserts no per-tile
    # reloads (~1.3us each).
    tables = list(get_activation_tables(nc.m.arch).keys())
    nc.scalar.add_instruction(mybir.InstLoadActFuncSet(
        name=nc.get_next_instruction_name(), ins=[], outs=[],
        act_func_set_id=tables.index("natural_log_exp_and_others")))

    const = ctx.enter_context(tc.tile_pool(name="const", bufs=1))
    identity_f32 = const.tile([P, P], F32)
    make_identity(nc, identity_f32[:])
    identity_f16 = const.tile([P, P], F16)
    make_identity(nc, identity_f16[:])
    mem_bf = const.tile([P, KC, C], F16)    # [slot_in_chunk, slot_chunk, C]
    memT_bf = const.tile([P, CC, K], F16)   # [c_in_chunk, c_chunk, slot]
    match_sb = const.tile([P, N_TILES], F32)

    # ---- preamble: normalize memory, build mem_bf and its transpose ----
    with tc.tile_pool(name="pre", bufs=1) as pre, \
         tc.tile_pool(name="pre_ps", bufs=2, space="PSUM") as pre_ps:
        mem_f32 = pre.tile([P, KC, C], F32)
        mem_re = mem_ap.rearrange("(j p) c -> p j c", p=P)
        for j in range(KC):
            nc.sync.dma_start(mem_f32[:, j], mem_re[:, j])
        r2m = pre.tile([P, KC], F32)
        sq_scr = pre.tile([P, C], F32)
        for j in range(KC):
            nc.scalar.activation(sq_scr[:], mem_f32[:, j], AF.Square,
                                 accum_out=r2m[:, j:j + 1])
        lnr = pre.tile([P, KC], F32)
        nc.scalar.activation(lnr[:], r2m[:], AF.Ln)
        rnrm = pre.tile([P, KC], F32)
        nc.scalar.activation(rnrm[:], lnr[:], AF.Exp, scale=-0.5)
        for j in range(KC):
            nc.vector.tensor_scalar_mul(mem_bf[:, j], mem_f32[:, j],
                                        rnrm[:, j:j + 1])
        for cc in range(CC):
            for j in range(KC):
                tp = pre_ps.tile([P, P], F16)
                nc.tensor.transpose(tp[:], mem_bf[:, j, ds(cc * P, P)],
                                    identity_f16[:])
                nc.scalar.copy(memT_bf[:, cc, ds(j * P, P)], tp[:])

    # ---- main loop pools ----
    x_pool = ctx.enter_context(tc.tile_pool(name="x", bufs=4))
    xbf_pool = ctx.enter_context(tc.tile_pool(name="xbf", bufs=4))
    attn_pool = ctx.enter_context(tc.tile_pool(name="attn", bufs=3))
    attnT_pool = ctx.enter_context(tc.tile_pool(name="attnT", bufs=3))
    out_pool = ctx.enter_context(tc.tile_pool(name="out", bufs=4))
    small = ctx.enter_context(tc.tile_pool(name="small", bufs=4))
    scr_pool = ctx.enter_context(tc.tile_pool(name="scr", bufs=2))
    # PSUM budget (8 banks): gram 1 + reconT 1 + sim 2x2 + attnT(f16) 1 + rc 1 = 8
    ps_gram = ctx.enter_context(tc.tile_pool(name="ps_gram", bufs=1, space="PSUM"))
    ps_sim = ctx.enter_context(tc.tile_pool(name="ps_sim", bufs=2, space="PSUM"))
    ps_trh = ctx.enter_context(tc.tile_pool(name="ps_trh", bufs=1, space="PSUM"))
    ps_rc = ctx.enter_context(tc.tile_pool(name="ps_rc", bufs=1, space="PSUM"))

    import contextlib
    loop_cm = (tc.For_i(0, reps, 1,
                        hint_engines=(mybir.EngineType.PE,
                                      mybir.EngineType.DVE,
                                      mybir.EngineType.Activation))
               if reps > 1 else contextlib.nullcontext())
    with loop_cm:
      for t in range(N_TILES):
        b, tb = divmod(t, TILES_PER_B)
        hw0 = tb * PT

        xt = x_pool.tile([P, CC, PT], F32, tag="xt")
        nc.sync.dma_start(
            xt[:], x_ap[b, :, ds(hw0, PT)].rearrange("(cc p) w -> p cc w", p=P))
        xbf = xbf_pool.tile([P, CC, PT], F16, tag="xbf")
        nc.gpsimd.tensor_copy(xbf[:], xt[:])

        # gram first: r2 -> rscale chain overlaps the sim matmuls below
        gram = ps_gram.tile([P, P], F32, tag="gram")
        for cc in range(CC):
            nc.tensor.matmul(gram[:], xbf[:, cc], xbf[:, cc],
                             start=(cc == 0), stop=(cc == CC - 1))
        r2 = small.tile([P, 1], F32, tag="r2")
        scr = scr_pool.tile([P, P], F32, tag="scr")
        nc.vector.tensor_tensor(scr[:], gram[:], identity_f32[:], ALU.mult)
        nc.vector.tensor_reduce(r2[:], scr[:], axis=AX.X, op=ALU.add)
        # rscale = r2^-0.5 = exp(-0.5 ln r2); Ln/Exp share one ACT table set
        lnr2 = small.tile([P, 1], F32, tag="lnr2")
        nc.scalar.activation(lnr2[:], r2[:], AF.Ln)
        rs = small.tile([P, 1], F32, tag="rs")
        nc.scalar.activation(rs[:], lnr2[:], AF.Exp, scale=-0.5)

        sim = ps_sim.tile([P, K], F32, tag="sim")
        for cc in range(CC):
            first, last = cc == 0, cc == CC - 1
            nc.tensor.matmul(sim[:, ds(0, 512)], xbf[:, cc],
                             memT_bf[:, cc, ds(0, 512)], start=first, stop=last)
            nc.tensor.matmul(sim[:, ds(512, 512)], xbf[:, cc],
                             memT_bf[:, cc, ds(512, 512)], start=first, stop=last)

        nmx = small.tile([P, 1], F32, tag="nmx")
        nc.vector.tensor_reduce(nmx[:], sim[:], axis=AX.X, op=ALU.max,
                                negate=True)
        negb = small.tile([P, 1], F32, tag="negb")
        nc.scalar.mul(negb[:], nmx[:], rs[:])

        attnN = attn_pool.tile([P, K], F16, tag="attnN")
        zsum = small.tile([P, 1], F32, tag="zsum")
        nc.scalar.activation(attnN[:], sim[:], AF.Exp,
                             bias=negb[:], scale=rs[:], accum_out=zsum[:])

        # transpose attn to slot-major: attnT[s, j, m]
        attnT = attnT_pool.tile([P, KC, PT], F16, tag="attnT")
        pat = ps_trh.tile([P, K], F16, tag="trh")
        for j in range(KC):
            nc.tensor.transpose(pat[:, ds(j * P, P)],
                                attnN[:, ds(j * P, P)], identity_f16[:])
        cp = pat.rearrange("p (j w) -> p j w", w=PT)
        nc.vector.tensor_copy(attnT[:, 0:4, :], cp[:, 0:4])
        nc.scalar.copy(attnT[:, 4:8, :], cp[:, 4:8])

        # MM2: recon_raw[m, c] = sum_s attnT[s, m] * mem_bf[s, c]
        prc = ps_rc.tile([P, C], F32, tag="prc")
        for j in range(KC):
            nc.tensor.matmul(prc[:], attnT[:, j], mem_bf[:, j],
                             start=(j == 0), stop=(j == KC - 1))

        # match_map tile = 1/Z ; also the recon row scale
        nc.vector.reciprocal(match_sb[:, t:t + 1], zsum[:])
        rc = out_pool.tile([P, C], F32, tag="rc")
        nc.vector.tensor_scalar_mul(rc[:], prc[:], match_sb[:, t:t + 1])

        # transpose recon to C-major and store (f32 to keep output precision)
        prt = ps_gram.tile([P, C], F32, tag="prt", name="prt")
        for cc in range(CC):
            nc.tensor.transpose(prt[:, ds(cc * P, P)], rc[:, ds(cc * P, P)],
                                identity_f32[:])
        rt = out_pool.tile([P, CC, P], F32, tag="rt")
        nc.vector.tensor_copy(rt[:], prt.rearrange("p (c w) -> p c w", w=P))
        nc.sync.dma_start(
            recon_ap[b, :, ds(hw0, PT)].rearrange("(cc p) w -> p cc w", p=P),
            rt[:])

    # ---- epilogue: transpose match_sb [P, N_TILES] -> [N_TILES, P], store ----
    mps = ps_rc.tile([P, C], F32, tag="prc", name="mps")
    nc.tensor.transpose(mps[:N_TILES, :P], match_sb[:], identity_f32[:])
    mt = out_pool.tile([N_TILES, P], F32, tag="mt")
    nc.vector.tensor_copy(mt[:], mps[:N_TILES, :P])
    for b in range(B_CORE):
        nc.sync.dma_start(
            match_ap[b].rearrange("(t p) -> t p", p=P),
            mt[ds(b * TILES_PER_B, TILES_PER_B), :])


_CACHE = {}


def build_nc(reps=1):
    if reps in _CACHE:
        return _CACHE[reps]
    nc = bacc.Bacc("TRN2", target_bir_lowering=False, debug=False,
                   num_devices=N_CORES)
    x_ap = nc.dram_tensor("x_s", [B_CORE, C, HW], F32, kind="ExternalInput").ap()
    mem_ap = nc.dram_tensor("mem", [K, C], F32, kind="ExternalInput").ap()
    recon_ap = nc.dram_tensor("recon_s", [B_CORE, C, HW], F32,
                              kind="ExternalOutput").ap()
    match_ap = nc.dram_tensor("match_s", [B_CORE, HW], F32,
                              kind="ExternalOutput").ap()
    with tile.TileContext(nc) as tc, ExitStack() as ctx:
        _emit(tc, ctx, x_ap, mem_ap, recon_ap, match_ap, reps=reps)
    nc.compile()
    _CACHE[reps] = nc
    return nc


def make_in_maps(x, memory):
    x = np.ascontiguousarray(np.asarray(x, dtype=np.float32)).reshape(
        N_CORES, B_CORE, C, HW)
    memory = np.ascontiguousarray(np.asarray(memory, dtype=np.float32))
    return [{"x_s": x[i], "mem": memory} for i in range(N_CORES)]


def kernel(x, memory):
    nc = build_nc()
    in_maps = make_in_maps(x, memory)
    res = run_bass_kernel_spmd(nc, in_maps, core_ids=list(range(N_CORES)))
    recon = np.stack([res.results[i]["recon_s"] for i in range(N_CORES)])
    match = np.stack([res.results[i]["match_s"] for i in range(N_CORES)])
    recon = recon.reshape(B, C, H, W).astype(np.float32)
    match = match.reshape(B, H, W).astype(np.float32)
    return recon, match


# revision 15
# speedup vs baseline: 1.1864x; 1.1640x over previous
"""MemoryBank2D (vq_codebook) Trainium2 kernel — 8-core data-parallel.

reference semantics:
    feat = x.transpose(0,2,3,1).reshape(-1, C)           # [N, C], N = B*H*W
    sim = l2norm(feat) @ l2norm(memory).T                # [N, K]
    attn = softmax(sim, axis=1)
    recon = (attn @ l2norm(memory)) -> [B, C, H, W]
    match_map = attn.max(axis=1)    -> [B, H, W]

Sharding: data-parallel over batch. 16 batches / 8 cores = 2 per core;
memory replicated; no collectives. Each core computes its recon/match
shard; the host concatenates.

Per-core kernel, per 128-patch tile (patches on PSUM partitions; x's
native [B, C, HW] layout IS the lhsT layout MM1 needs, so no input
transpose):
  - cast x tile f32->f16 on GPSIMD (idle engine)
  - gram matmul (rhs = x tile itself, f16) FIRST: its diagonal is
    ||feat||^2; the rscale = r2^-0.5 chain (DVE diag reduce -> ACT
    Ln -> ACT Exp(scale=-0.5)) overlaps the sim matmuls
  - MM1 f16: sim_raw = x^T @ memT_n, fp32 PSUM accumulation over 4
    C-chunks (2 N=512 matmuls per chunk)
  - softmax: negated rowmax on DVE; ONE ACT Exp pass computes
    attn = exp(sim_raw*rscale - rowmax*rscale) in f16 AND its row sum Z
    via accum_out. match_map = 1/Z exactly (max attn numerator = exp(0)).
    All ACT funcs (Ln/Exp/Copy) live in one pre-seeded table set
    ("natural_log_exp_and_others") => no per-tile 1.3us table reloads.
  - PE-transpose attn (f16, 1 cyc/row) into one PSUM bank -> attnT
  - MM2 f16: recon_raw = attnT.T @ mem_n (8 slot-chunks, N=512);
    scale rows by 1/Z; PE-transpose recon (f32) to C-major so the DRAM
    write has 512B-contiguous runs
PSUM budget exactly 8 banks: gram 1 + reconT 1 + sim 2x2 + attnT 1 + rc 1.

Measured (8 NeuronCores, axon): ~350 us per core-pass (slope of wall
time vs in-NEFF hardware-loop reps; cost model: 325 us, PE busy 289 us
vs 221 us pure-matmul floor). Accuracy vs f32 reference (absmax-rel):
recon ~1.7e-4, match ~5.7e-5.
"""

import numpy as np
from contextlib import ExitStack

import concourse.bass as bass
import concourse.tile as tile
from concourse import bacc, mybir
from concourse.bass import ds
from concourse.bass_utils import run_bass_kernel_spmd
from concourse.masks import make_identity
from concourse.hw_specs import get_activation_tables

F32 = mybir.dt.float32
BF16 = mybir.dt.bfloat16
F16 = mybir.dt.float16
AF = mybir.ActivationFunctionType
ALU = mybir.AluOpType
AX = mybir.AxisListType

# problem shapes (hardcoded per contract)
B, C, H, W = 16, 512, 64, 64
K = 1024
N_CORES = 8
B_CORE = B // N_CORES          # 2
HW = H * W                     # 4096
P = 128
CC = C // P                    # 4 contraction chunks
KC = K // P                    # 8 slot chunks
PT = 128                       # patches per tile
TILES_PER_B = HW // PT         # 32
N_TILES = B_CORE * TILES_PER_B  # 64


def _emit(tc, ctx, x_ap, mem_ap, recon_ap, match_ap, reps=1):
    nc = tc.nc

    # Pre-load the one ACT table set covering every function used here
    # (Ln, Exp, Copy, Square) so the table-load pass inserts no per-tile
    # reloads (~1.3us each).
    tables = list(get_activation_tables(nc.m.arch).keys())
    nc.scalar.add_instruction(mybir.InstLoadActFuncSet(
        name=nc.get_next_instruction_name(), ins=[], outs=[],
        act_func_set_id=tables.index("natural_log_exp_and_others")))

    const = ctx.enter_context(tc.tile_pool(name="const", bufs=1))
    identity_f32 = const.tile([P, P], F32)
    make_identity(nc, identity_f32[:])
    identity_f16 = const.tile([P, P], F16)
    make_identity(nc, identity_f16[:])
    mem_bf = const.tile([P, KC, C], F16)    # [slot_in_chunk, slot_chunk, C]
    memT_bf = const.tile([P, CC, K], F16)   # [c_in_chunk, c_chunk, slot]
    match_sb = const.tile([P, N_TILES], F32)

    # ---- preamble: normalize memory, build mem_bf and its transpose ----
    with tc.tile_pool(name="pre", bufs=1) as pre, \
         tc.tile_pool(name="pre_ps", bufs=2, space="PSUM") as pre_ps:
        mem_f32 = pre.tile([P, KC, C], F32)
        mem_re = mem_ap.rearrange("(j p) c -> p j c", p=P)
        for j in range(KC):
            nc.sync.dma_start(mem_f32[:, j], mem_re[:, j])
        r2m = pre.tile([P, KC], F32)
        sq_scr = pre.tile([P, C], F32)
        for j in range(KC):
            nc.scalar.activation(sq_scr[:], mem_f32[:, j], AF.Square,
                                 accum_out=r2m[:, j:j + 1])
        lnr = pre.tile([P, KC], F32)
        nc.scalar.activation(lnr[:], r2m[:], AF.Ln)
        rnrm = pre.tile([P, KC], F32)
        nc.scalar.activation(rnrm[:], lnr[:], AF.Exp, scale=-0.5)
        for j in range(KC):
            nc.vector.tensor_scalar_mul(mem_bf[:, j], mem_f32[:, j],
                                        rnrm[:, j:j + 1])
        for cc in range(CC):
            for j in range(KC):
                tp = pre_ps.tile([P, P], F16)
                nc.tensor.transpose(tp[:], mem_bf[:, j, ds(cc * P, P)],
                                    identity_f16[:])
                nc.scalar.copy(memT_bf[:, cc, ds(j * P, P)], tp[:])

    # ---- main loop pools ----
    x_pool = ctx.enter_context(tc.tile_pool(name="x", bufs=4))
    xbf_pool = ctx.enter_context(tc.tile_pool(name="xbf", bufs=4))
    attn_pool = ctx.enter_context(tc.tile_pool(name="attn", bufs=3))
    attnT_pool = ctx.enter_context(tc.tile_pool(name="attnT", bufs=3))
    out_pool = ctx.enter_context(tc.tile_pool(name="out", bufs=4))
    small = ctx.enter_context(tc.tile_pool(name="small", bufs=4))
    scr_pool = ctx.enter_context(tc.tile_pool(name="scr", bufs=2))
    # PSUM budget (8 banks): gram 1 + reconT 1 + sim 2x2 + attnT(f16) 1 + rc 1 = 8
    ps_gram = ctx.enter_context(tc.tile_pool(name="ps_gram", bufs=1, space="PSUM"))
    ps_sim = ctx.enter_context(tc.tile_pool(name="ps_sim", bufs=2, space="PSUM"))
    ps_trh = ctx.enter_context(tc.tile_pool(name="ps_trh", bufs=1, space="PSUM"))
    ps_rc = ctx.enter_context(tc.tile_pool(name="ps_rc", bufs=1, space="PSUM"))

    import contextlib
    loop_cm = (tc.For_i(0, reps, 1,
                        hint_engines=(mybir.EngineType.PE,
                                      mybir.EngineType.DVE,
                                      mybir.EngineType.Activation))
               if reps > 1 else contextlib.nullcontext())
    with loop_cm:
      for t in range(N_TILES):
        b, tb = divmod(t, TILES_PER_B)
        hw0 = tb * PT

        xt = x_pool.tile([P, CC, PT], F32, tag="xt")
        nc.sync.dma_start(
            xt[:], x_ap[b, :, ds(hw0, PT)].rearrange("(cc p) w -> p cc w", p=P))
        xbf = xbf_pool.tile([P, CC, PT], F16, tag="xbf")
        nc.gpsimd.tensor_copy(xbf[:], xt[:])

        # gram first: r2 -> rscale chain overlaps the sim matmuls below
        gram = ps_gram.tile([P, P], F32, tag="gram")
        for cc in range(CC):
            nc.tensor.matmul(gram[:], xbf[:, cc], xbf[:, cc],
                             start=(cc == 0), stop=(cc == CC - 1))
        r2 = small.tile([P, 1], F32, tag="r2")
        scr = scr_pool.tile([P, P], F32, tag="scr")
        nc.vector.tensor_tensor(scr[:], gram[:], identity_f32[:], ALU.mult)
        nc.vector.tensor_reduce(r2[:], scr[:], axis=AX.X, op=ALU.add)
        # rscale = r2^-0.5 = exp(-0.5 ln r2); Ln/Exp share one ACT table set
        lnr2 = small.tile([P, 1], F32, tag="lnr2")
        nc.scalar.activation(lnr2[:], r2[:], AF.Ln)
        rs = small.tile([P, 1], F32, tag="rs")
        nc.scalar.activation(rs[:], lnr2[:], AF.Exp, scale=-0.5)

        sim = ps_sim.tile([P, K], F32, tag="sim")
        for cc in range(CC):
            first, last = cc == 0, cc == CC - 1
            nc.tensor.matmul(sim[:, ds(0, 512)], xbf[:, cc],
                             memT_bf[:, cc, ds(0, 512)], start=first, stop=last)
            nc.tensor.matmul(sim[:, ds(512, 512)], xbf[:, cc],
                             memT_bf[:, cc, ds(512, 512)], start=first, stop=last)

        # split rowmax over the two PSUM halves: the first half's reduce
        # starts one matmul earlier, shortening the chain into the Exp
        nmx0 = small.tile([P, 1], F32, tag="nmx0")
        nc.vector.tensor_reduce(nmx0[:], sim[:, ds(0, 512)], axis=AX.X,
                                op=ALU.max, negate=True)
        nmx = small.tile([P, 1], F32, tag="nmx")
        nc.vector.tensor_reduce(nmx[:], sim[:, ds(512, 512)], axis=AX.X,
                                op=ALU.max, negate=True)
        nc.vector.tensor_tensor(nmx[:], nmx0[:], nmx[:], ALU.min)
        negb = small.tile([P, 1], F32, tag="negb")
        nc.scalar.mul(negb[:], nmx[:], rs[:])

        attnN = attn_pool.tile([P, K], F16, tag="attnN")
        zsum = small.tile([P, 1], F32, tag="zsum")
        nc.scalar.activation(attnN[:], sim[:], AF.Exp,
                             bias=negb[:], scale=rs[:], accum_out=zsum[:])

        # transpose attn to slot-major: attnT[s, j, m]
        attnT = attnT_pool.tile([P, KC, PT], F16, tag="attnT")
        pat = ps_trh.tile([P, K], F16, tag="trh")
        for j in range(KC):
            nc.tensor.transpose(pat[:, ds(j * P, P)],
                                attnN[:, ds(j * P, P)], identity_f16[:])
        cp = pat.rearrange("p (j w) -> p j w", w=PT)
        nc.vector.tensor_copy(attnT[:, 0:4, :], cp[:, 0:4])
        nc.scalar.copy(attnT[:, 4:8, :], cp[:, 4:8])

        # MM2: recon_raw[m, c] = sum_s attnT[s, m] * mem_bf[s, c]
        prc = ps_rc.tile([P, C], F32, tag="prc")
        for j in range(KC):
            nc.tensor.matmul(prc[:], attnT[:, j], mem_bf[:, j],
                             start=(j == 0), stop=(j == KC - 1))

        # match_map tile = 1/Z ; also the recon row scale
        nc.vector.reciprocal(match_sb[:, t:t + 1], zsum[:])
        rc = out_pool.tile([P, C], F32, tag="rc")
        nc.vector.tensor_scalar_mul(rc[:], prc[:], match_sb[:, t:t + 1])

        # transpose recon to C-major and store (f32 to keep output precision)
        prt = ps_gram.tile([P, C], F32, tag="prt", name="prt")
        for cc in range(CC):
            nc.tensor.transpose(prt[:, ds(cc * P, P)], rc[:, ds(cc * P, P)],
                                identity_f32[:])
        rt = out_pool.tile([P, CC, P], F32, tag="rt")
        nc.vector.tensor_copy(rt[:], prt.rearrange("p (c w) -> p c w", w=P))
        nc.sync.dma_start(
            recon_ap[b, :, ds(hw0, PT)].rearrange("(cc p) w -> p cc w", p=P),
            rt[:])

    # ---- epilogue: transpose match_sb [P, N_TILES] -> [N_TILES, P], store ----
    mps = ps_rc.tile([P, C], F32, tag="prc", name="mps")
    nc.tensor.transpose(mps[:N_TILES, :P], match_sb[:], identity_f32[:])
    mt = out_pool.tile([N_TILES, P], F32, tag="mt")
    nc.vector.tensor_copy(mt[:], mps[:N_TILES, :P])
    for b in range(B_CORE):
        nc.sync.dma_start(
            match_ap[b].rearrange("(t p) -> t p", p=P),
            mt[ds(b * TILES_PER_B, TILES_PER_B), :])


_CACHE = {}


def build_nc(reps=1):
    if reps in _CACHE:
        return _CACHE[reps]
    nc = bacc.Bacc("TRN2", target_bir_lowering=False, debug=False,
                   num_devices=N_CORES)
    x_ap = nc.dram_tensor("x_s", [B_CORE, C, HW], F32, kind="ExternalInput").ap()
    mem_ap = nc.dram_tensor("mem", [K, C], F32, kind="ExternalInput").ap()
    recon_ap = nc.dram_tensor("recon_s", [B_CORE, C, HW], F32,
                              kind="ExternalOutput").ap()
    match_ap = nc.dram_tensor("match_s", [B_CORE, HW], F32,
                              kind="ExternalOutput").ap()
    with tile.TileContext(nc) as tc, ExitStack() as ctx:
        _emit(tc, ctx, x_ap, mem_ap, recon_ap, match_ap, reps=reps)
    nc.compile()
    _CACHE[reps] = nc
    return nc


def make_in_maps(x, memory):
    x = np.ascontiguousarray(np.asarray(x, dtype=np.float32)).reshape(
        N_CORES, B_CORE, C, HW)
    memory = np.ascontiguousarray(np.asarray(memory, dtype=np.float32))
    return [{"x_s": x[i], "mem": memory} for i in range(N_CORES)]


def kernel(x, memory):
    nc = build_nc()
    in_maps = make_in_maps(x, memory)
    res = run_bass_kernel_spmd(nc, in_maps, core_ids=list(range(N_CORES)))
    recon = np.stack([res.results[i]["recon_s"] for i in range(N_CORES)])
    match = np.stack([res.results[i]["match_s"] for i in range(N_CORES)])
    recon = recon.reshape(B, C, H, W).astype(np.float32)
    match = match.reshape(B, H, W).astype(np.float32)
    return recon, match


# revision 19
# speedup vs baseline: 1.4332x; 1.2080x over previous
"""MemoryBank2D (vq_codebook) Trainium2 kernel — 8-core data-parallel.

reference semantics:
    feat = x.transpose(0,2,3,1).reshape(-1, C)           # [N, C], N = B*H*W
    sim = l2norm(feat) @ l2norm(memory).T                # [N, K]
    attn = softmax(sim, axis=1)
    recon = (attn @ l2norm(memory)) -> [B, C, H, W]
    match_map = attn.max(axis=1)    -> [B, H, W]

Sharding: data-parallel over batch. 16 batches / 8 cores = 2 per core;
memory replicated; no collectives. Each core computes its recon/match
shard; the host concatenates.

Per-core kernel, per 128-patch tile (patches on PSUM partitions; x's
native [B, C, HW] layout IS the lhsT layout MM1 needs, so no input
transpose):
  - cast x tile f32->f16 on GPSIMD (idle engine)
  - gram matmul (rhs = x tile itself, f16) FIRST: its diagonal is
    ||feat||^2; the rscale = r2^-0.5 chain (DVE diag reduce -> ACT
    Ln -> ACT Exp(scale=-0.5)) overlaps the sim matmuls
  - MM1 f16: sim_raw = x^T @ memT_n, fp32 PSUM accumulation over 4
    C-chunks (2 N=512 matmuls per chunk)
  - softmax: negated rowmax on DVE; ONE ACT Exp pass computes
    attn = exp(sim_raw*rscale - rowmax*rscale) in f16 AND its row sum Z
    via accum_out. match_map = 1/Z exactly (max attn numerator = exp(0)).
    All ACT funcs (Ln/Exp/Copy) live in one pre-seeded table set
    ("natural_log_exp_and_others") => no per-tile 1.3us table reloads.
  - PE-transpose attn (f16, 1 cyc/row) into one PSUM bank -> attnT
  - MM2 f16: recon_raw = attnT.T @ mem_n (8 slot-chunks, N=512);
    scale rows by 1/Z; PE-transpose recon (f32) to C-major so the DRAM
    write has 512B-contiguous runs
PSUM budget exactly 8 banks: gram 1 + reconT 1 + sim 2x2 + attnT 1 + rc 1.

Measured (8 NeuronCores, axon): ~333 us per core-pass (LSQ slope of
wall time vs in-NEFF hardware-loop reps; cost model: 319 us, PE busy
289 us vs 221 us pure-matmul floor; steady-state PE gaps ~0 in the
model). Accuracy vs f32 reference (absmax-rel): recon ~1.7e-4,
match ~4.5e-5.
"""

import numpy as np
from contextlib import ExitStack

import concourse.bass as bass
import concourse.tile as tile
from concourse import bacc, mybir
from concourse.bass import ds
from concourse.bass_utils import run_bass_kernel_spmd
from concourse.masks import make_identity
from concourse.hw_specs import get_activation_tables

F32 = mybir.dt.float32
BF16 = mybir.dt.bfloat16
F16 = mybir.dt.float16
AF = mybir.ActivationFunctionType
ALU = mybir.AluOpType
AX = mybir.AxisListType

# problem shapes (hardcoded per contract)
B, C, H, W = 16, 512, 64, 64
K = 1024
N_CORES = 8
B_CORE = B // N_CORES          # 2
HW = H * W                     # 4096
P = 128
CC = C // P                    # 4 contraction chunks
KC = K // P                    # 8 slot chunks
PT = 128                       # patches per tile
TILES_PER_B = HW // PT         # 32
N_TILES = B_CORE * TILES_PER_B  # 64


def _emit(tc, ctx, x_ap, mem_ap, recon_ap, match_ap, reps=1):
    nc = tc.nc

    # Pre-load the one ACT table set covering every function used here
    # (Ln, Exp, Copy, Square) so the table-load pass inserts no per-tile
    # reloads (~1.3us each).
    tables = list(get_activation_tables(nc.m.arch).keys())
    nc.scalar.add_instruction(mybir.InstLoadActFuncSet(
        name=nc.get_next_instruction_name(), ins=[], outs=[],
        act_func_set_id=tables.index("natural_log_exp_and_others")))

    const = ctx.enter_context(tc.tile_pool(name="const", bufs=1))
    identity_f32 = const.tile([P, P], F32)
    make_identity(nc, identity_f32[:])
    identity_f16 = const.tile([P, P], F16)
    make_identity(nc, identity_f16[:])
    mem_bf = const.tile([P, KC, C], F16)    # [slot_in_chunk, slot_chunk, C]
    memT_bf = const.tile([P, CC, K], F16)   # [c_in_chunk, c_chunk, slot]
    match_sb = const.tile([P, N_TILES], F32)

    # ---- preamble: normalize memory, build mem_bf and its transpose ----
    with tc.tile_pool(name="pre", bufs=1) as pre, \
         tc.tile_pool(name="pre_ps", bufs=2, space="PSUM") as pre_ps:
        mem_f32 = pre.tile([P, KC, C], F32)
        mem_re = mem_ap.rearrange("(j p) c -> p j c", p=P)
        for j in range(KC):
            nc.sync.dma_start(mem_f32[:, j], mem_re[:, j])
        r2m = pre.tile([P, KC], F32)
        sq_scr = pre.tile([P, C], F32)
        for j in range(KC):
            nc.scalar.activation(sq_scr[:], mem_f32[:, j], AF.Square,
                                 accum_out=r2m[:, j:j + 1])
        lnr = pre.tile([P, KC], F32)
        nc.scalar.activation(lnr[:], r2m[:], AF.Ln)
        rnrm = pre.tile([P, KC], F32)
        nc.scalar.activation(rnrm[:], lnr[:], AF.Exp, scale=-0.5)
        for j in range(KC):
            nc.vector.tensor_scalar_mul(mem_bf[:, j], mem_f32[:, j],
                                        rnrm[:, j:j + 1])
        for cc in range(CC):
            for j in range(KC):
                tp = pre_ps.tile([P, P], F16)
                nc.tensor.transpose(tp[:], mem_bf[:, j, ds(cc * P, P)],
                                    identity_f16[:])
                nc.scalar.copy(memT_bf[:, cc, ds(j * P, P)], tp[:])

    # ---- main loop pools ----
    x_pool = ctx.enter_context(tc.tile_pool(name="x", bufs=4))
    xbf_pool = ctx.enter_context(tc.tile_pool(name="xbf", bufs=4))
    attn_pool = ctx.enter_context(tc.tile_pool(name="attn", bufs=3))
    attnT_pool = ctx.enter_context(tc.tile_pool(name="attnT", bufs=3))
    out_pool = ctx.enter_context(tc.tile_pool(name="out", bufs=4))
    small = ctx.enter_context(tc.tile_pool(name="small", bufs=4))
    scr_pool = ctx.enter_context(tc.tile_pool(name="scr", bufs=2))
    # PSUM budget (8 banks): gram 1 + reconT 1 + sim 2x2 + attnT(f16) 1 + rc 1 = 8
    ps_gram = ctx.enter_context(tc.tile_pool(name="ps_gram", bufs=1, space="PSUM"))
    ps_sim = ctx.enter_context(tc.tile_pool(name="ps_sim", bufs=2, space="PSUM"))
    ps_trh = ctx.enter_context(tc.tile_pool(name="ps_trh", bufs=1, space="PSUM"))
    ps_rc = ctx.enter_context(tc.tile_pool(name="ps_rc", bufs=1, space="PSUM"))

    import contextlib
    loop_cm = (tc.For_i(0, reps, 1,
                        hint_engines=(mybir.EngineType.PE,
                                      mybir.EngineType.DVE,
                                      mybir.EngineType.Activation))
               if reps > 1 else contextlib.nullcontext())
    with loop_cm:
      for t in range(N_TILES):
        b, tb = divmod(t, TILES_PER_B)
        hw0 = tb * PT

        xt = x_pool.tile([P, CC, PT], F32, tag="xt")
        nc.sync.dma_start(
            xt[:], x_ap[b, :, ds(hw0, PT)].rearrange("(cc p) w -> p cc w", p=P))
        xbf = xbf_pool.tile([P, CC, PT], F16, tag="xbf")
        nc.gpsimd.tensor_copy(xbf[:], xt[:])

        # gram first: r2 -> rscale chain overlaps the sim matmuls below
        gram = ps_gram.tile([P, P], F32, tag="gram")
        for cc in range(CC):
            nc.tensor.matmul(gram[:], xbf[:, cc], xbf[:, cc],
                             start=(cc == 0), stop=(cc == CC - 1))
        r2 = small.tile([P, 1], F32, tag="r2")
        scr = scr_pool.tile([P, P], F32, tag="scr")
        nc.vector.tensor_tensor(scr[:], gram[:], identity_f32[:], ALU.mult)
        nc.vector.tensor_reduce(r2[:], scr[:], axis=AX.X, op=ALU.add)
        # rscale = r2^-0.5 = exp(-0.5 ln r2); Ln/Exp share one ACT table set
        lnr2 = small.tile([P, 1], F32, tag="lnr2")
        nc.scalar.activation(lnr2[:], r2[:], AF.Ln)
        rs = small.tile([P, 1], F32, tag="rs")
        nc.scalar.activation(rs[:], lnr2[:], AF.Exp, scale=-0.5)

        sim = ps_sim.tile([P, K], F32, tag="sim")
        for cc in range(CC):
            first, last = cc == 0, cc == CC - 1
            nc.tensor.matmul(sim[:, ds(0, 512)], xbf[:, cc],
                             memT_bf[:, cc, ds(0, 512)], start=first, stop=last)
            nc.tensor.matmul(sim[:, ds(512, 512)], xbf[:, cc],
                             memT_bf[:, cc, ds(512, 512)], start=first, stop=last)

        # split rowmax over the two PSUM halves: the first half's reduce
        # starts one matmul earlier, shortening the chain into the Exp
        nmx0 = small.tile([P, 1], F32, tag="nmx0")
        nc.vector.tensor_reduce(nmx0[:], sim[:, ds(0, 512)], axis=AX.X,
                                op=ALU.max, negate=True)
        nmx = small.tile([P, 1], F32, tag="nmx")
        nc.vector.tensor_reduce(nmx[:], sim[:, ds(512, 512)], axis=AX.X,
                                op=ALU.max, negate=True)
        nc.vector.tensor_tensor(nmx[:], nmx0[:], nmx[:], ALU.min)
        negb = small.tile([P, 1], F32, tag="negb")
        nc.scalar.mul(negb[:], nmx[:], rs[:])

        attnN = attn_pool.tile([P, K], F16, tag="attnN")
        zsum = small.tile([P, 1], F32, tag="zsum")
        nc.scalar.activation(attnN[:], sim[:], AF.Exp,
                             bias=negb[:], scale=rs[:], accum_out=zsum[:])

        # transpose attn to slot-major: attnT[s, j, m]
        attnT = attnT_pool.tile([P, KC, PT], F16, tag="attnT")
        pat = ps_trh.tile([P, K], F16, tag="trh")
        for j in range(KC):
            nc.tensor.transpose(pat[:, ds(j * P, P)],
                                attnN[:, ds(j * P, P)], identity_f16[:])
        cp = pat.rearrange("p (j w) -> p j w", w=PT)
        nc.vector.tensor_copy(attnT[:, 0:4, :], cp[:, 0:4])
        nc.scalar.copy(attnT[:, 4:8, :], cp[:, 4:8])

        # MM2: recon_raw[m, c] = sum_s attnT[s, m] * mem_bf[s, c]
        prc = ps_rc.tile([P, C], F32, tag="prc")
        for j in range(KC):
            nc.tensor.matmul(prc[:], attnT[:, j], mem_bf[:, j],
                             start=(j == 0), stop=(j == KC - 1))

        # match_map tile = 1/Z ; also the recon row scale
        nc.vector.reciprocal(match_sb[:, t:t + 1], zsum[:])
        rc = out_pool.tile([P, C], F32, tag="rc")
        nc.vector.tensor_scalar_mul(rc[:], prc[:], match_sb[:, t:t + 1])

        # transpose recon to C-major and store (f32 to keep output precision)
        prt = ps_gram.tile([P, C], F32, tag="prt", name="prt")
        for cc in range(CC):
            nc.tensor.transpose(prt[:, ds(cc * P, P)], rc[:, ds(cc * P, P)],
                                identity_f32[:])
        rt = out_pool.tile([P, CC, P], F32, tag="rt")
        nc.vector.tensor_copy(rt[:], prt.rearrange("p (c w) -> p c w", w=P))
        nc.sync.dma_start(
            recon_ap[b, :, ds(hw0, PT)].rearrange("(cc p) w -> p cc w", p=P),
            rt[:])

    # ---- epilogue: transpose match_sb [P, N_TILES] -> [N_TILES, P], store ----
    mps = ps_rc.tile([P, C], F32, tag="prc", name="mps")
    nc.tensor.transpose(mps[:N_TILES, :P], match_sb[:], identity_f32[:])
    mt = out_pool.tile([N_TILES, P], F32, tag="mt")
    nc.vector.tensor_copy(mt[:], mps[:N_TILES, :P])
    for b in range(B_CORE):
        nc.sync.dma_start(
            match_ap[b].rearrange("(t p) -> t p", p=P),
            mt[ds(b * TILES_PER_B, TILES_PER_B), :])


_CACHE = {}


def build_nc(reps=1):
    if reps in _CACHE:
        return _CACHE[reps]
    nc = bacc.Bacc("TRN2", target_bir_lowering=False, debug=False,
                   num_devices=N_CORES)
    x_ap = nc.dram_tensor("x_s", [B_CORE, C, HW], F32, kind="ExternalInput").ap()
    mem_ap = nc.dram_tensor("mem", [K, C], F32, kind="ExternalInput").ap()
    recon_ap = nc.dram_tensor("recon_s", [B_CORE, C, HW], F32,
                              kind="ExternalOutput").ap()
    match_ap = nc.dram_tensor("match_s", [B_CORE, HW], F32,
                              kind="ExternalOutput").ap()
    with tile.TileContext(nc) as tc, ExitStack() as ctx:
        _emit(tc, ctx, x_ap, mem_ap, recon_ap, match_ap, reps=reps)
    nc.compile()
    _CACHE[reps] = nc
    return nc


def make_in_maps(x, memory):
    x = np.ascontiguousarray(np.asarray(x, dtype=np.float32)).reshape(
        N_CORES, B_CORE, C, HW)
    memory = np.ascontiguousarray(np.asarray(memory, dtype=np.float32))
    return [{"x_s": x[i], "mem": memory} for i in range(N_CORES)]


def kernel(x, memory):
    nc = build_nc()
    in_maps = make_in_maps(x, memory)
    res = run_bass_kernel_spmd(nc, in_maps, core_ids=list(range(N_CORES)))
    recon = np.stack([res.results[i]["recon_s"] for i in range(N_CORES)])
    match = np.stack([res.results[i]["match_s"] for i in range(N_CORES)])
    recon = recon.reshape(B, C, H, W).astype(np.float32)
    match = match.reshape(B, H, W).astype(np.float32)
    return recon, match
